# revision 13
# baseline (speedup 1.0000x reference)
"""Trainium2 Bass kernel for the DifferentiableMPO mirror-descent solve.

Math (per (b,h) pair, all fp32):
    Sigma = L @ L.T  (precomputed once on device)
    300 fixed iterations of:
        A      = Sigma @ w            (the reference's L @ (L.T @ w))
        mu_p   = mu . w
        ssq    = w . A                (= ||L.T w||^2)
        sigma  = ssq * (1/sqrt(ssq))
        viol   = -mu_p + KAPPA*sigma - cvar
        sg     = sigmoid(50*viol)
        g      = -(1+50*sg)*mu + (2*GAMMA + 50*KAPPA*sg/sigma)*A
                 + 1e-3*(s_h - s_{h+1}),   s = diff/sqrt(diff^2+1e-8)
        w      = softmax(log(w+1e-10) - 0.05*g)

Layout ("transposed world"): everything lives as [N(part) x pairs(free)] with
pairs ordered h-major (p = h*16 + b) per core.  One matmul per pair per
iteration with stationary lhsT_p = [Sigma_p | mu_p] ([100 x 101]) and moving
w[:, p] -> outputs pack as adjacent PSUM columns (no extraction needed).
Reductions over i use ones-matmuls; per-pair scalars broadcast back across
partitions via K=1 matmuls.

Sharding: pure data parallel, batch dim 128 -> 16 per core across 8 cores.
"""

import numpy as np

B, H, N = 128, 12, 100
NCORES = 8
BC = B // NCORES          # 16 batches per core
P = H * BC                # 192 pairs per core, p = h*BC + b
SC = N                    # stationary cols per pair (Sigma only)
ITERS = 300
NCHUNK = 2
CW = P // NCHUNK          # 96 pairs per chunk

GAMMA = 5.0
COST = 1e-3
KAPPA = 2.0627128
CP = 50.0                 # CVAR_PENALTY
SBETA = 50.0              # SOFTPLUS_BETA
ETA = 0.05


def build_nc(iters=ITERS, nc=None, compile=True):
    import concourse.bass as bass
    import concourse.mybir as mybir
    import concourse.tile as tile
    from concourse import bacc, masks

    f32 = mybir.dt.float32
    AF = mybir.ActivationFunctionType
    OP = mybir.AluOpType
    if nc is None:
        nc = bacc.Bacc(None, target_bir_lowering=False, debug=False)

    mu_d = nc.dram_tensor("mu", [BC, H, N], f32, kind="ExternalInput")
    L_d = nc.dram_tensor("L", [BC, H, N, N], f32, kind="ExternalInput")
    wp_d = nc.dram_tensor("w_prev", [BC, N], f32, kind="ExternalInput")
    cv_d = nc.dram_tensor("cvar", [1, BC], f32, kind="ExternalInput")
    out_d = nc.dram_tensor("w_out", [BC, H, N], f32, kind="ExternalOutput")

    with tile.TileContext(nc) as tc, (
            tc.tile_pool(name="const", bufs=1)) as cpool, (
            tc.tile_pool(name="loop", bufs=2)) as lpool, (
            tc.tile_pool(name="small", bufs=4)) as rpool:
        stat = cpool.tile([N, P * SC], f32)      # per-pair [Sigma | mu]
        muT = cpool.tile([N, P], f32)
        wprevT = cpool.tile([N, BC], f32)
        cvrow = cpool.tile([1, P], f32)
        ones_col = cpool.tile([N, 1], f32)       # lhsT for sum-over-i
        ones_row = cpool.tile([1, N], f32)       # lhsT for bcast (K=1)
        ident = cpool.tile([128, 128], f32)
        cbias = cpool.tile([128, 5], f32)        # bias constants per column

        masks.make_identity(nc, ident[:])
        nc.vector.memset(ones_col[:], 1.0)
        nc.vector.memset(ones_row[:], 1.0)
        for col, val in enumerate([0.0, 1e-8, 1e-10, 1.0, 2.0 * GAMMA]):
            nc.vector.memset(cbias[:, col:col + 1], val)
        for h in range(H):
            nc.sync.dma_start(cvrow[:, h * BC:(h + 1) * BC], cv_d[:])

        with (
            tc.tile_pool(name="stage", bufs=3) as spool,
            tc.tile_pool(name="pre_ps", bufs=2, space="PSUM") as prepsum,
        ):
            # muT[:, p] = mu[b, h, :] with p = h*16+b, via 2 staged transposes
            for half in range(2):
                mst = spool.tile([96, N], f32, tag="mst")
                for hh in range(6):
                    h = 6 * half + hh
                    nc.sync.dma_start(mst[hh * BC:(hh + 1) * BC, :],
                                      mu_d[:, h, :])
                pT = prepsum.tile([N, 96], f32, tag="ptr")
                nc.tensor.transpose(pT[:], mst[:], ident[0:96, 0:96])
                nc.vector.tensor_copy(muT[:, 96 * half:96 * (half + 1)], pT[:])

            wst = spool.tile([BC, N], f32, tag="wst")
            nc.sync.dma_start(wst[:], wp_d[:])
            pT = prepsum.tile([N, BC], f32, tag="ptr")
            nc.tensor.transpose(pT[:], wst[:], ident[0:BC, 0:BC])
            nc.vector.tensor_copy(wprevT[:], pT[:])

            # per-pair Sigma = L @ L.T into stat cols, plus mu column
            for p in range(P):
                h, b = divmod(p, BC)
                Lt = spool.tile([N, N], f32, tag="Lt")
                nc.sync.dma_start(Lt[:], L_d[b, h])
                pLT = prepsum.tile([N, N], f32, tag="pLT")
                nc.tensor.transpose(pLT[:], Lt[:], ident[0:N, 0:N])
                LTs = spool.tile([N, N], f32, tag="LTs")
                nc.vector.tensor_copy(LTs[:], pLT[:])
                pS = prepsum.tile([N, N], f32, tag="pSg")
                nc.tensor.matmul(pS[:], LTs[:], LTs[:])
                nc.scalar.copy(stat[:, p * SC:p * SC + N], pS[:])

        with (
            tc.tile_pool(name="psA", bufs=3, space="PSUM") as psA,
            tc.tile_pool(name="psS", bufs=2, space="PSUM") as psS,
            tc.tile_pool(name="psB", bufs=3, space="PSUM") as psB,
        ):
            w_cur = lpool.tile([N, P], f32, tag="w")
            nc.vector.memset(w_cur[:], 1.0 / N)

            for _k in range(iters):
                # ln(w + 1e-10): first ACT op, shares the ln/exp table set
                # with the previous iteration's Exp -> no table swap here
                lw = lpool.tile([N, P], f32, tag="lw")
                nc.scalar.activation(lw[:], w_cur[:], AF.Ln,
                                     bias=cbias[0:N, 2:3])

                # cost-gradient part, depends only on w_cur
                diff = lpool.tile([N, P], f32, tag="diff")
                nc.vector.tensor_sub(diff[:, BC:], w_cur[:, BC:],
                                     w_cur[:, 0:P - BC])
                nc.vector.tensor_sub(diff[:, 0:BC], w_cur[:, 0:BC], wprevT[:])
                sq = lpool.tile([N, P], f32, tag="sq")
                nc.vector.tensor_mul(sq[:], diff[:], diff[:])
                rt = lpool.tile([N, P], f32, tag="rt")
                nc.scalar.activation(rt[:], sq[:], AF.Sqrt, bias=cbias[0:N, 1:2])
                rc = lpool.tile([N, P], f32, tag="rc")
                nc.vector.reciprocal(rc[:], rt[:])
                s = lpool.tile([N, P], f32, tag="s")
                nc.vector.tensor_mul(s[:], diff[:], rc[:])
                gc = lpool.tile([N, P], f32, tag="gc")
                nc.vector.tensor_sub(gc[:, 0:P - BC], s[:, 0:P - BC], s[:, BC:])
                nc.vector.tensor_copy(gc[:, P - BC:P], s[:, P - BC:P])

                # per-chunk matvecs + fused (ssq | mu_p) ones-reduction
                pAs, pSl = [], []
                for c in range(NCHUNK):
                    cs, ce = c * CW, (c + 1) * CW
                    pA = psA.tile([N, CW], f32, tag="pA")
                    for j in range(CW):
                        p = cs + j
                        nc.tensor.matmul(pA[:, j:j + 1],
                                         stat[:, p * SC:(p + 1) * SC],
                                         w_cur[:, p:p + 1])
                    A = lpool.tile([N, CW], f32, tag="A" + str(c))
                    nc.vector.tensor_copy(A[:], pA[:])
                    prod = lpool.tile([N, 2 * CW], f32, tag="prod")
                    nc.vector.tensor_mul(prod[:, 0:CW], w_cur[:, cs:ce], A[:])
                    nc.vector.tensor_mul(prod[:, CW:], muT[:, cs:ce],
                                         w_cur[:, cs:ce])
                    pSs = psS.tile([1, 2 * CW], f32, tag="pSm")
                    nc.tensor.matmul(pSs[:], ones_col[:], prod[:])
                    pAs.append(A)
                    pSl.append(pSs)

                # batched per-pair scalar stage on [1, P] rows
                ssqrow = rpool.tile([1, P], f32, tag="ssqrow")
                murow = rpool.tile([1, P], f32, tag="murow")
                for c in range(NCHUNK):
                    nc.vector.tensor_copy(ssqrow[:, c * CW:(c + 1) * CW],
                                          pSl[c][:, 0:CW])
                    nc.vector.tensor_copy(murow[:, c * CW:(c + 1) * CW],
                                          pSl[c][:, CW:])
                sigp = rpool.tile([1, P], f32, tag="sigp")
                nc.scalar.activation(sigp[:], ssqrow[:], AF.Sqrt,
                                     bias=cbias[0:1, 0:1])
                rsig = rpool.tile([1, P], f32, tag="rsig")
                nc.vector.reciprocal(rsig[:], sigp[:])
                v0 = rpool.tile([1, P], f32, tag="v0")
                nc.vector.scalar_tensor_tensor(
                    v0[:], sigp[:], KAPPA, murow[:],
                    op0=OP.mult, op1=OP.subtract)
                viol = rpool.tile([1, P], f32, tag="viol")
                nc.vector.tensor_sub(viol[:], v0[:], cvrow[:])
                # sigmoid(50*v) = 1 / (1 + exp(-50*v)) -- Exp shares the
                # ln/exp table set, avoiding a Sigmoid table load
                sgh = rpool.tile([1, P], f32, tag="sgh")
                nc.scalar.activation(sgh[:], viol[:], AF.Exp, scale=-SBETA,
                                     bias=cbias[0:1, 0:1])
                den = rpool.tile([1, P], f32, tag="den")
                nc.vector.tensor_scalar_add(den[:], sgh[:], 1.0)
                sg = rpool.tile([1, P], f32, tag="sg")
                nc.vector.reciprocal(sg[:], den[:])
                alpha = rpool.tile([1, P], f32, tag="alpha")
                nc.vector.tensor_scalar(alpha[:], sg[:], CP, 1.0,
                                        op0=OP.mult, op1=OP.add)
                t2 = rpool.tile([1, P], f32, tag="t2")
                nc.vector.tensor_mul(t2[:], sg[:], rsig[:])
                beta = rpool.tile([1, P], f32, tag="beta")
                nc.vector.tensor_scalar(beta[:], t2[:], CP * KAPPA,
                                        2.0 * GAMMA, op0=OP.mult, op1=OP.add)

                w_next = lpool.tile([N, P], f32, tag="w")

                for c in range(NCHUNK):
                    cs, ce = c * CW, (c + 1) * CW
                    A = pAs[c]
                    pB1 = psB.tile([N, 2 * CW], f32, tag="pB")
                    nc.tensor.matmul(pB1[:, 0:CW], ones_row[:],
                                     beta[:, cs:ce])
                    nc.tensor.matmul(pB1[:, CW:], ones_row[:],
                                     alpha[:, cs:ce])
                    gA = lpool.tile([N, CW], f32, tag="gA")
                    nc.vector.tensor_mul(gA[:], A[:], pB1[:, 0:CW])
                    gM = lpool.tile([N, CW], f32, tag="gM")
                    nc.vector.tensor_mul(gM[:], muT[:, cs:ce], pB1[:, CW:])
                    g0 = lpool.tile([N, CW], f32, tag="g0")
                    nc.vector.tensor_sub(g0[:], gA[:], gM[:])
                    g1 = lpool.tile([N, CW], f32, tag="g1")
                    nc.vector.scalar_tensor_tensor(
                        g1[:], gc[:, cs:ce], COST, g0[:],
                        op0=OP.mult, op1=OP.add)
                    logit = lpool.tile([N, CW], f32, tag="logit")
                    nc.vector.scalar_tensor_tensor(
                        logit[:], g1[:], -ETA, lw[:, cs:ce],
                        op0=OP.mult, op1=OP.add)
                    E = lpool.tile([N, CW], f32, tag="E")
                    nc.scalar.activation(E[:], logit[:], AF.Exp,
                                         bias=cbias[0:N, 0:1])
                    pS2 = psS.tile([1, CW], f32, tag="pSm")
                    nc.tensor.matmul(pS2[:], ones_col[:], E[:])
                    rS = rpool.tile([1, CW], f32, tag="rS")
                    nc.vector.reciprocal(rS[:], pS2[:])
                    pB2 = psB.tile([N, CW], f32, tag="pB")
                    nc.tensor.matmul(pB2[:], ones_row[:], rS[:])
                    nc.vector.tensor_mul(w_next[:, cs:ce], E[:], pB2[:])

                w_cur = w_next

            # write out: transpose back to [pairs x N] then DMA per h
            for half in range(2):
                pT = psB.tile([96, N], f32, tag="pB")
                nc.tensor.transpose(pT[:], w_cur[:, 96 * half:96 * (half + 1)],
                                    ident[0:N, 0:N])
                ost = lpool.tile([96, N], f32, tag="ost")
                nc.vector.tensor_copy(ost[:], pT[:])
                for hh in range(6):
                    h = 6 * half + hh
                    nc.sync.dma_start(out_d[:, h, :],
                                      ost[hh * BC:(hh + 1) * BC, :])
    if compile:
        nc.compile()
    return nc


def kernel(mu, L, w_prev, cvar_limit):
    from concourse.bass_utils import run_bass_kernel_spmd

    nc = build_nc(ITERS)
    in_maps = []
    for c in range(NCORES):
        sl = slice(c * BC, (c + 1) * BC)
        in_maps.append({
            "mu": np.ascontiguousarray(mu[sl], dtype=np.float32),
            "L": np.ascontiguousarray(L[sl], dtype=np.float32),
            "w_prev": np.ascontiguousarray(w_prev[sl], dtype=np.float32),
            "cvar": np.ascontiguousarray(
                cvar_limit[sl], dtype=np.float32).reshape(1, BC),
        })
    res = run_bass_kernel_spmd(nc, in_maps, list(range(NCORES)))
    out = np.concatenate([res.results[c]["w_out"] for c in range(NCORES)],
                         axis=0)
    return out.astype(np.float32)
